# revision 6
# baseline (speedup 1.0000x reference)
"""Attention with host-folded QK^T kernel + pair-wise V dedup AllGather.

v5: the K projection never runs on device. scores = q·k^T with
q = x W_q, k = x W_k factors as x (W_q W_k^T) x^T, so the host
precomputes M = 64·W_q W_k^T (fp32 matmul, then fp16 — the 64×
scale keeps M's ~1e-5-magnitude entries out of fp16 subnormals;
the exp activation scale absorbs the 1/64). Each core computes
Q' = x_q M for its query half (same cost as the old Q projection)
and scores come from Q'·x_k^T against the xh slabs directly —
the entire 32µs redundant full-K projection is gone.

V keeps the pair-dedup: each core computes V' for its own
sequence half, pairs exchange halves with one 2-rank AllGather
(physical = rank order, zero rank-dependent addressing), and the
gather's deadline is the out phase (~100µs in), far beyond the
~80µs the CC path needs (23µs CC boot barrier + ~11µs op setup +
~23µs transfer).

Schedule notes:
- A1 V' runs 512/256-wide with ps1/ps2 interleaved per dc so
  consecutive matmuls share the stationary slab chunk (weight
  load elides).
- psa keeps one PSUM bank unused so phase B's first score psum
  doesn't wait on phase A's last drain.
- out phase: denominator run (cols 512:770) first, so the recip
  and the cols-512:768 normalize overlap the cols-0:512 run.
- wv/qslab0 (the A1-critical inputs) split across the sync and
  gpsimd queues to land together.
"""

import numpy as np

import concourse.bass as bass
import concourse.mybir as mybir
import concourse.tile as tile
from concourse import bacc
from concourse.bass_utils import run_bass_kernel_spmd

N_CORES = 8
B, N, D, OUT = 4, 2048, 768, 768
NQ = N // 2
P = 128
DC = D // P
KC = N // P
HKC = KC // 2  # k-chunks per half
F32 = mybir.dt.float32
FP16 = mybir.dt.float16
PAIRS = [[0, 1], [2, 3], [4, 5], [6, 7]]

M_SCALE = 64.0  # host folds this into M; exp scale divides it back out


def build_attention_nc():
    nc = bacc.Bacc("TRN2", target_bir_lowering=False, debug=False)
    xh = nc.dram_tensor("xh", [D, N], FP16, kind="ExternalInput")
    xq = nc.dram_tensor("xq", [D, NQ], FP16, kind="ExternalInput")
    mw = nc.dram_tensor("mw", [D, D], FP16, kind="ExternalInput")
    wvi = nc.dram_tensor("wvi", [D, OUT], FP16, kind="ExternalInput")
    out = nc.dram_tensor("out", [NQ, OUT], F32, kind="ExternalOutput")

    with tile.TileContext(nc) as tc:
        with (
            tc.tile_pool(name="persist", bufs=1) as persist,
            tc.tile_pool(name="slabs", bufs=6) as slabs,
            tc.tile_pool(name="dpool", bufs=1, space="DRAM") as dpool,
        ):
            # Q'^T[d,q], one tile per 512-query half so the scores phase
            # never waits on the other half's psum drain
            qpt = [
                persist.tile([P, DC, 512], FP16, name=f"qpt{s}")
                for s in range(2)
            ]
            vp = persist.tile([P, KC, OUT + 2], FP16)  # V' physical order

            vpb_in = dpool.tile([P, HKC, OUT + 2], FP16)
            vpb_out = dpool.tile([2, P, HKC, OUT + 2], FP16)

            ones_sc = persist.tile([P, 1], F32, name="ones_sc")
            nc.vector.memset(ones_sc, 1.0)
            zero_sc = persist.tile([P, 1], F32, name="zero_sc")
            nc.vector.memset(zero_sc, 0.0)

            with (
                tc.tile_pool(name="psa", bufs=6, space="PSUM") as psa,
                tc.tile_pool(name="wpool", bufs=1) as wpool,
                tc.tile_pool(name="stage", bufs=4) as stage,
            ):
                wv_sb = wpool.tile([P, DC, OUT], FP16)
                mw_sb = wpool.tile([P, DC, D], FP16)

                # HAM warmup while the first DMAs fly
                warm = wpool.tile([P, 512], FP16, name="warm")
                nc.vector.memset(warm, 1.0)
                wps = psa.tile([P, 512], F32, name="wps", bufs=1)
                for i in range(11):
                    nc.tensor.matmul(
                        wps, warm[:, 0:P], warm, start=(i == 0), stop=(i == 10)
                    )

                # DMAs: wv/xq-slab0 first (A1-critical), balanced across
                # the sync and gpsimd queues; then xq-slab1, M, xh slabs
                qslab_tiles = []
                for s in range(2):
                    qslab = slabs.tile(
                        [P, DC, 512], FP16, tag="slab", name=f"qslab{s}"
                    )
                    src = xq[:, s * 512 : (s + 1) * 512]
                    if s == 0:
                        for dc in range(DC):
                            nc.gpsimd.dma_start(
                                out=wv_sb[:, dc, :],
                                in_=wvi[dc * P : (dc + 1) * P, :],
                            )
                            eng = nc.sync if dc % 2 == 0 else nc.scalar
                            eng.dma_start(
                                out=qslab[:, dc, :],
                                in_=src[dc * P : (dc + 1) * P, :],
                            )
                    else:
                        nc.sync.dma_start(
                            out=qslab,
                            in_=src.rearrange("(dc p) n -> p dc n", p=P),
                        )
                    qslab_tiles.append(qslab)
                for dc in range(DC):
                    nc.sync.dma_start(
                        out=mw_sb[:, dc, :], in_=mw[dc * P : (dc + 1) * P, :]
                    )
                kslab_tiles = []
                for s in range(4):
                    kslab = slabs.tile(
                        [P, DC, 512], FP16, tag="slab", name=f"kslab{s}"
                    )
                    nc.sync.dma_start(
                        out=kslab,
                        in_=xh[:, s * 512 : (s + 1) * 512].rearrange(
                            "(dc p) n -> p dc n", p=P
                        ),
                    )
                    kslab_tiles.append(kslab)

                # ---- A1: V' half (earliest -> feeds the gather) ----
                # 512/256-wide runs; ps1/ps2 interleaved per dc so both
                # matmuls share the stationary token chunk (weight load
                # elides on the second)
                for kc in range(8):
                    slab = qslab_tiles[kc // 4]
                    j = kc % 4
                    ps1 = psa.tile([P, 512], F32, tag="psa")
                    ps2 = psa.tile([P, 512], F32, tag="psa")
                    for dc in range(DC):
                        nc.tensor.matmul(
                            ps1,
                            slab[:, dc, j * P : (j + 1) * P],
                            wv_sb[:, dc, 0:512],
                            start=(dc == 0),
                            stop=(dc == DC - 1),
                        )
                        nc.tensor.matmul(
                            ps2[:, 0:256],
                            slab[:, dc, j * P : (j + 1) * P],
                            wv_sb[:, dc, 512:OUT],
                            start=(dc == 0),
                            stop=(dc == DC - 1),
                        )
                    vst = stage.tile([P, OUT + 2], FP16, tag="vst", bufs=9)
                    nc.vector.tensor_copy(vst[:, 0:512], ps1)
                    nc.vector.tensor_copy(vst[:, 512:OUT], ps2[:, 0:256])
                    nc.vector.tensor_copy(vst[:, OUT : OUT + 1], ones_sc)
                    nc.vector.tensor_copy(vst[:, OUT + 1 : OUT + 2], zero_sc)
                    nc.gpsimd.dma_start(out=vpb_in[:, kc, :], in_=vst)
                nc.gpsimd.collective_compute(
                    "AllGather",
                    mybir.AluOpType.bypass,
                    replica_groups=PAIRS,
                    ins=[vpb_in.opt()],
                    outs=[vpb_out.opt()],
                )
                # NOT on the scalar ring: the ACT sequencer is busy with
                # the exp activations by the time the gather lands, and the
                # readback would queue behind them (measured 4.4us stall)
                for h in range(2):
                    nc.sync.dma_start(
                        out=vp[:, h * HKC : (h + 1) * HKC, :], in_=vpb_out[h]
                    )

                # ---- A2: Q'^T = (x_q M)^T half (local) ----
                for s in range(2):
                    slab = qslab_tiles[s]
                    for oc in range(DC):
                        ps = psa.tile([P, 512], F32, tag="psa")
                        for dc in range(DC):
                            nc.tensor.matmul(
                                ps,
                                mw_sb[:, dc, oc * P : (oc + 1) * P],
                                slab[:, dc, :],
                                start=(dc == 0),
                                stop=(dc == DC - 1),
                            )
                        nc.vector.tensor_copy(qpt[s][:, oc, :], ps)

            # ---- phase B: all scoresT runs, then all out runs ----
            # scoresT contracts over d: stationary = xh slab chunks,
            # moving = Q'^T. No K tensor exists on device.
            with (
                tc.tile_pool(name="expp", bufs=34) as expp,
                tc.tile_pool(name="obp", bufs=3) as obp,
                tc.tile_pool(name="smallp", bufs=4) as smallp,
                tc.tile_pool(name="ps_sc", bufs=2, space="PSUM") as ps_sc,
                tc.tile_pool(name="ps_out", bufs=3, space="PSUM") as ps_out,
            ):
                ets = {}
                for bi in range(2):
                    for kc in range(KC):
                        kslab = kslab_tiles[kc // 4]
                        c0 = (kc % 4) * P
                        st = ps_sc.tile([P, 512], F32, tag="sc")
                        for dc in range(DC):
                            nc.tensor.matmul(
                                st,
                                kslab[:, dc, c0 : c0 + P],
                                qpt[bi][:, dc, :],
                                start=(dc == 0),
                                stop=(dc == DC - 1),
                            )
                        et = expp.tile(
                            [P, 512], FP16, tag="exp", name=f"et{bi}_{kc}"
                        )
                        nc.scalar.activation(
                            et,
                            st,
                            mybir.ActivationFunctionType.Exp,
                            scale=0.125 / M_SCALE,
                        )
                        ets[(bi, kc)] = et
                # out runs: 8 q-chunks of 128, rotating 3 PSUM bufs.
                # Denominator run (cols 512:770) goes FIRST so the recip
                # and the 512:768 normalize overlap the 0:512 run; the
                # post work is emitted per chunk so the tail stays short.
                for j in range(NQ // P):
                    bi, jj = j // 4, j % 4
                    ops = ps_out.tile(
                        [P, OUT + 2], F32, tag="out", name=f"outps{j}"
                    )
                    for kc in range(KC):
                        nc.tensor.matmul(
                            ops[:, 512 : OUT + 2],
                            ets[(bi, kc)][:, jj * P : (jj + 1) * P],
                            vp[:, kc, 512 : OUT + 2],
                            start=(kc == 0),
                            stop=(kc == KC - 1),
                        )
                    recip = smallp.tile([P, 1], F32, tag="recip")
                    nc.vector.reciprocal(recip, ops[:, OUT : OUT + 1])
                    ob = obp.tile([P, OUT], F32, tag="ob")
                    nc.vector.tensor_scalar_mul(
                        ob[:, 512:OUT], ops[:, 512:OUT], recip
                    )
                    for kc in range(KC):
                        nc.tensor.matmul(
                            ops[:, 0:512],
                            ets[(bi, kc)][:, jj * P : (jj + 1) * P],
                            vp[:, kc, 0:512],
                            start=(kc == 0),
                            stop=(kc == KC - 1),
                        )
                    nc.vector.tensor_scalar_mul(
                        ob[:, 0:512], ops[:, 0:512], recip
                    )
                    nc.sync.dma_start(
                        out=out[j * P : (j + 1) * P, :], in_=ob
                    )
    nc.finalize()
    return nc


_NC_CACHE = None


def _get_nc():
    global _NC_CACHE
    if _NC_CACHE is None:
        _NC_CACHE = build_attention_nc()
    return _NC_CACHE


def make_in_maps(x, kernel):
    x = np.asarray(x, dtype=np.float32)
    w = np.asarray(kernel, dtype=np.float32)
    mw = np.ascontiguousarray(
        (M_SCALE * (w[0] @ w[1].T)).astype(np.float16)
    )
    wv = np.ascontiguousarray(w[2].astype(np.float16))
    in_maps = []
    for core in range(N_CORES):
        b, half = core // 2, core % 2
        xt16 = x[b].T.astype(np.float16)
        xh = np.ascontiguousarray(xt16)
        xq = np.ascontiguousarray(xt16[:, half * NQ : (half + 1) * NQ])
        in_maps.append({"xh": xh, "xq": xq, "mw": mw, "wvi": wv})
    return in_maps


def assemble_output(results):
    out = np.empty((B, N, OUT), dtype=np.float32)
    for core in range(N_CORES):
        b, half = core // 2, core % 2
        out[b, half * NQ : (half + 1) * NQ, :] = results[core]["out"]
    return out


def run_on_hw(x, kernel, trace=False):
    nc = _get_nc()
    res = run_bass_kernel_spmd(
        nc, make_in_maps(x, kernel), list(range(N_CORES)), trace=trace
    )
    return assemble_output(res.results), res


def kernel(x, kernel):
    out, _ = run_on_hw(x, kernel, trace=False)
    return out
